# revision 12
# baseline (speedup 1.0000x reference)
"""ContrastLoss kernel for Trainium2 (8 NeuronCores, SPMD data-parallel).

loss = -sum_i dot(f_s[i], f_t[i]) / B  ==  -sum(f_s * f_t) / B

The row structure is irrelevant: the answer is the global sum of the
elementwise product. Each core gets 1/8 of the batch (a flat 4M-element
chunk viewed as [128, 32768]), computes per-partition partial sums with
fused DVE tensor_tensor_reduce ops, and the host sums the 8x[128 x T]
partials and applies -1/B.
"""

import sys

for _p in (
    "/root/.axon_site",
    "/root/.axon_site/_ro/trn_rl_repo",
    "/root/.axon_site/_ro/pypackages",
    "/opt/trn_rl_repo",
    "/opt/pypackages",
):
    if _p not in sys.path:
        sys.path.append(_p)

import numpy as np

B, D = 65536, 512
N_CORES = 8
P = 128
ROWS_PER_CORE = B // N_CORES              # 8192
FREE = ROWS_PER_CORE * D // P             # 32768 f32 per partition per tensor
# Tile column widths: big tiles amortize DMA overhead; the shrinking tail
# keeps the last mult+reduce chain (which runs after DMA goes idle) short.
TILE_SCHEDULE = [4096] * 7 + [2048, 2048]
assert sum(TILE_SCHEDULE) == FREE
N_TILES = len(TILE_SCHEDULE)

_CACHE = {}


def _build():
    from contextlib import ExitStack

    import concourse.bacc as bacc
    import concourse.mybir as mybir
    import concourse.tile as tile

    nc = bacc.Bacc(
        "TRN2", target_bir_lowering=False, debug=False, num_devices=N_CORES
    )
    a = nc.declare_dram_parameter("a", [P, FREE], mybir.dt.float32, isOutput=False)
    b = nc.declare_dram_parameter("b", [P, FREE], mybir.dt.float32, isOutput=False)
    out = nc.declare_dram_parameter(
        "partials", [P, N_TILES], mybir.dt.float32, isOutput=True
    )

    with tile.TileContext(nc) as tc, ExitStack() as ctx:
        pa = ctx.enter_context(tc.tile_pool(name="pa", bufs=3))
        pb = ctx.enter_context(tc.tile_pool(name="pb", bufs=3))
        pm = ctx.enter_context(tc.tile_pool(name="pm", bufs=2))
        pacc = ctx.enter_context(tc.tile_pool(name="pacc", bufs=1))

        acc = pacc.tile([P, N_TILES], mybir.dt.float32)
        col = 0
        for t, tile_n in enumerate(TILE_SCHEDULE):
            sl = slice(col, col + tile_n)
            col += tile_n
            ta = pa.tile([P, tile_n], mybir.dt.float32, tag="ta")
            nc.sync.dma_start(out=ta[:], in_=a[:, sl])
            tb = pb.tile([P, tile_n], mybir.dt.float32, tag="tb")
            nc.scalar.dma_start(out=tb[:], in_=b[:, sl])
            tm = pm.tile([P, tile_n], mybir.dt.float32, tag="tm")
            nc.vector.tensor_mul(tm[:], ta[:], tb[:])
            tj = pm.tile([P, tile_n], mybir.dt.float32, tag="junk")
            nc.scalar.activation(
                out=tj[:],
                in_=tm[:],
                func=mybir.ActivationFunctionType.Copy,
                accum_out=acc[:, t : t + 1],
            )
        nc.sync.dma_start(out=out[:], in_=acc[:])
    nc.compile()
    return nc


def _get_nc():
    if "nc" not in _CACHE:
        _CACHE["nc"] = _build()
    return _CACHE["nc"]


def run(f_s, f_t, trace=False):
    """Returns (loss ndarray shape (1,) f32, exec_time_ns or None)."""
    from concourse.bass_utils import run_bass_kernel_spmd

    f_s = np.ascontiguousarray(np.asarray(f_s, dtype=np.float32))
    f_t = np.ascontiguousarray(np.asarray(f_t, dtype=np.float32))
    assert f_s.shape == (B, D) and f_t.shape == (B, D)

    in_maps = []
    for c in range(N_CORES):
        rows = slice(c * ROWS_PER_CORE, (c + 1) * ROWS_PER_CORE)
        in_maps.append(
            {
                "a": f_s[rows].reshape(P, FREE),
                "b": f_t[rows].reshape(P, FREE),
            }
        )

    res = run_bass_kernel_spmd(_get_nc(), in_maps, list(range(N_CORES)), trace=trace)
    _CACHE["last_results"] = res
    total = np.float64(0.0)
    for r in res.results:
        total += r["partials"].astype(np.float64).sum()
    loss = np.asarray([-total / B], dtype=np.float32)
    return loss, res.exec_time_ns


def kernel(f_s, f_t):
    return run(f_s, f_t, trace=False)[0]


# revision 14
# speedup vs baseline: 1.0341x; 1.0341x over previous
"""ContrastLoss kernel for Trainium2 (8 NeuronCores, SPMD data-parallel).

loss = -sum_i dot(f_s[i], f_t[i]) / B  ==  -sum(f_s * f_t) / B

The row structure is irrelevant: the answer is the global sum of the
elementwise product. Each core gets 1/8 of the batch (a flat 4M-element
chunk viewed as [128, 32768]), computes per-partition partial sums with
fused DVE tensor_tensor_reduce ops, and the host sums the 8x[128 x T]
partials and applies -1/B.
"""

import sys

for _p in (
    "/root/.axon_site",
    "/root/.axon_site/_ro/trn_rl_repo",
    "/root/.axon_site/_ro/pypackages",
    "/opt/trn_rl_repo",
    "/opt/pypackages",
):
    if _p not in sys.path:
        sys.path.append(_p)

import numpy as np

B, D = 65536, 512
N_CORES = 8
P = 128
ROWS_PER_CORE = B // N_CORES              # 8192
FREE = ROWS_PER_CORE * D // P             # 32768 f32 per partition per tensor
# Tile column widths: big tiles amortize DMA overhead; the shrinking tail
# keeps the last mult+reduce chain (which runs after DMA goes idle) short.
TILE_SCHEDULE = [4096] * 7 + [2048, 2048]
assert sum(TILE_SCHEDULE) == FREE
N_TILES = len(TILE_SCHEDULE)

_CACHE = {}


def _build():
    from contextlib import ExitStack

    import concourse.bacc as bacc
    import concourse.mybir as mybir
    import concourse.tile as tile

    nc = bacc.Bacc(
        "TRN2", target_bir_lowering=False, debug=False, num_devices=N_CORES
    )
    a = nc.declare_dram_parameter("a", [P, FREE], mybir.dt.float32, isOutput=False)
    b = nc.declare_dram_parameter("b", [P, FREE], mybir.dt.float32, isOutput=False)
    out = nc.declare_dram_parameter(
        "partials", [P, N_TILES], mybir.dt.float32, isOutput=True
    )

    with tile.TileContext(nc) as tc, ExitStack() as ctx:
        pa = ctx.enter_context(tc.tile_pool(name="pa", bufs=4))
        pb = ctx.enter_context(tc.tile_pool(name="pb", bufs=4))
        pm = ctx.enter_context(tc.tile_pool(name="pm", bufs=2))
        pacc = ctx.enter_context(tc.tile_pool(name="pacc", bufs=1))

        acc = pacc.tile([P, N_TILES], mybir.dt.float32)
        col = 0
        for t, tile_n in enumerate(TILE_SCHEDULE):
            sl = slice(col, col + tile_n)
            col += tile_n
            ta = pa.tile([P, tile_n], mybir.dt.float32, tag="ta")
            nc.sync.dma_start(out=ta[:], in_=a[:, sl])
            tb = pb.tile([P, tile_n], mybir.dt.float32, tag="tb")
            nc.sync.dma_start(out=tb[:], in_=b[:, sl])
            tm = pm.tile([P, tile_n], mybir.dt.float32, tag="tm")
            nc.vector.tensor_mul(tm[:], ta[:], tb[:])
            nc.scalar.activation(
                out=tm[:],
                in_=tm[:],
                func=mybir.ActivationFunctionType.Copy,
                accum_out=acc[:, t : t + 1],
            )
        nc.sync.dma_start(out=out[:], in_=acc[:])
    nc.compile()
    return nc


def _get_nc():
    if "nc" not in _CACHE:
        _CACHE["nc"] = _build()
    return _CACHE["nc"]


def run(f_s, f_t, trace=False):
    """Returns (loss ndarray shape (1,) f32, exec_time_ns or None)."""
    from concourse.bass_utils import run_bass_kernel_spmd

    f_s = np.ascontiguousarray(np.asarray(f_s, dtype=np.float32))
    f_t = np.ascontiguousarray(np.asarray(f_t, dtype=np.float32))
    assert f_s.shape == (B, D) and f_t.shape == (B, D)

    in_maps = []
    for c in range(N_CORES):
        rows = slice(c * ROWS_PER_CORE, (c + 1) * ROWS_PER_CORE)
        in_maps.append(
            {
                "a": f_s[rows].reshape(P, FREE),
                "b": f_t[rows].reshape(P, FREE),
            }
        )

    res = run_bass_kernel_spmd(_get_nc(), in_maps, list(range(N_CORES)), trace=trace)
    _CACHE["last_results"] = res
    total = np.float64(0.0)
    for r in res.results:
        total += r["partials"].astype(np.float64).sum()
    loss = np.asarray([-total / B], dtype=np.float32)
    return loss, res.exec_time_ns


def kernel(f_s, f_t):
    return run(f_s, f_t, trace=False)[0]


# revision 18
# speedup vs baseline: 1.2419x; 1.2009x over previous
"""ContrastLoss kernel for Trainium2 (8 NeuronCores, SPMD data-parallel).

loss = -sum_i dot(f_s[i], f_t[i]) / B  ==  -sum(f_s * f_t) / B

The row structure is irrelevant: the answer is the global sum of the
elementwise product. Each core gets 1/8 of the batch (a flat 4M-element
chunk viewed as [128, 32768]), computes per-partition partial sums
(DVE multiply -> ACT accumulate-reduce), and the host sums the
8 x [128 x T] partials and applies -1/B.
"""

import sys

for _p in (
    "/root/.axon_site",
    "/root/.axon_site/_ro/trn_rl_repo",
    "/root/.axon_site/_ro/pypackages",
    "/opt/trn_rl_repo",
    "/opt/pypackages",
):
    if _p not in sys.path:
        sys.path.append(_p)

import numpy as np

B, D = 65536, 512
N_CORES = 8
P = 128
ROWS_PER_CORE = B // N_CORES              # 8192
FREE = ROWS_PER_CORE * D // P             # 32768 f32 per partition per tensor

CONFIG = {
    # Tile column widths: big tiles amortize DMA descriptor overhead; the
    # shrinking tail keeps the final mult+reduce chain (after DMA idles) short.
    "schedule": [4096] * 7 + [2048, 1024, 512, 512],
    "bufs_ab": 3,
    "bufs_m": 2,
    "inplace_act": False,
    "issue": "sync",  # "sync" = one HWDGE ring, "dual" = a on sync, b on scalar
    # "split": a and b as two DRAM tensors, two DMAs per tile.
    # "interleaved": host packs [a_tile | b_tile] pairs into one flat DRAM
    # tensor; one DMA per tile, purely sequential HBM addresses.
    "layout": "split",
}

_CACHE = {}


def _build(cfg):
    from contextlib import ExitStack

    import concourse.bacc as bacc
    import concourse.mybir as mybir
    import concourse.tile as tile

    schedule = cfg["schedule"]
    assert sum(schedule) == FREE
    n_tiles = len(schedule)

    nc = bacc.Bacc(
        "TRN2", target_bir_lowering=False, debug=False, num_devices=N_CORES
    )
    interleaved = cfg["layout"] == "interleaved"
    if interleaved:
        ab = nc.declare_dram_parameter(
            "ab", [1, 2 * FREE * P], mybir.dt.float32, isOutput=False
        )
    else:
        a = nc.declare_dram_parameter(
            "a", [P, FREE], mybir.dt.float32, isOutput=False
        )
        b = nc.declare_dram_parameter(
            "b", [P, FREE], mybir.dt.float32, isOutput=False
        )
    out = nc.declare_dram_parameter(
        "partials", [P, n_tiles], mybir.dt.float32, isOutput=True
    )

    with tile.TileContext(nc) as tc, ExitStack() as ctx:
        pa = ctx.enter_context(tc.tile_pool(name="pa", bufs=cfg["bufs_ab"]))
        pb = ctx.enter_context(tc.tile_pool(name="pb", bufs=cfg["bufs_ab"]))
        pm = ctx.enter_context(tc.tile_pool(name="pm", bufs=cfg["bufs_m"]))
        pacc = ctx.enter_context(tc.tile_pool(name="pacc", bufs=1))

        b_engine = nc.scalar if cfg["issue"] == "dual" else nc.sync

        acc = pacc.tile([P, n_tiles], mybir.dt.float32)
        col = 0
        for t, tile_n in enumerate(schedule):
            if interleaved:
                off = 2 * col * P
                tab = pa.tile([P, 2 * tile_n], mybir.dt.float32, tag="ta")
                src = ab[:, off : off + 2 * tile_n * P].rearrange(
                    "1 (p n) -> p n", p=P
                )
                nc.sync.dma_start(out=tab[:], in_=src)
                in0, in1 = tab[:, :tile_n], tab[:, tile_n:]
            else:
                sl = slice(col, col + tile_n)
                ta = pa.tile([P, tile_n], mybir.dt.float32, tag="ta")
                nc.sync.dma_start(out=ta[:], in_=a[:, sl])
                tb = pb.tile([P, tile_n], mybir.dt.float32, tag="tb")
                b_engine.dma_start(out=tb[:], in_=b[:, sl])
                in0, in1 = ta[:], tb[:]
            col += tile_n
            tm = pm.tile([P, tile_n], mybir.dt.float32, tag="tm")
            nc.vector.tensor_mul(tm[:], in0, in1)
            if cfg["inplace_act"]:
                tj = tm
            else:
                tj = pm.tile([P, tile_n], mybir.dt.float32, tag="junk")
            nc.scalar.activation(
                out=tj[:],
                in_=tm[:],
                func=mybir.ActivationFunctionType.Copy,
                accum_out=acc[:, t : t + 1],
            )
        nc.sync.dma_start(out=out[:], in_=acc[:])
    nc.compile()
    return nc, n_tiles


def _get_nc():
    key = repr(sorted(CONFIG.items(), key=lambda kv: kv[0]))
    if key not in _CACHE:
        _CACHE[key] = _build(CONFIG)
    return _CACHE[key]


def run(f_s, f_t, trace=False):
    """Returns (loss ndarray shape (1,) f32, exec_time_ns or None)."""
    from concourse.bass_utils import run_bass_kernel_spmd

    f_s = np.ascontiguousarray(np.asarray(f_s, dtype=np.float32))
    f_t = np.ascontiguousarray(np.asarray(f_t, dtype=np.float32))
    assert f_s.shape == (B, D) and f_t.shape == (B, D)

    in_maps = []
    for c in range(N_CORES):
        rows = slice(c * ROWS_PER_CORE, (c + 1) * ROWS_PER_CORE)
        ac = f_s[rows].reshape(P, FREE)
        bc = f_t[rows].reshape(P, FREE)
        if CONFIG["layout"] == "interleaved":
            parts = []
            col = 0
            for w in CONFIG["schedule"]:
                parts.append(
                    np.concatenate([ac[:, col : col + w], bc[:, col : col + w]], axis=1)
                )
                col += w
            flat = np.concatenate([p.reshape(1, -1) for p in parts], axis=1)
            in_maps.append({"ab": flat})
        else:
            in_maps.append({"a": ac, "b": bc})

    nc, _ = _get_nc()
    res = run_bass_kernel_spmd(nc, in_maps, list(range(N_CORES)), trace=trace)
    _CACHE["last_results"] = res
    total = np.float64(0.0)
    for r in res.results:
        total += r["partials"].astype(np.float64).sum()
    loss = np.asarray([-total / B], dtype=np.float32)
    return loss, res.exec_time_ns


def kernel(f_s, f_t):
    return run(f_s, f_t, trace=False)[0]
